# revision 23
# baseline (speedup 1.0000x reference)
"""Trainium2 Bass kernel: separable box filter (radius 4) on (8,3,1024,1024) fp32.

v8: fp8 input, H-pass-first, batched DMA.

 - Host casts x to fp8 e4m3 (halves input HBM traffic; L2 rel err 3.0e-3 vs
   the 2e-2 budget, measured on the true jax key-0 input).  Output fp16.
 - Inputs land as ONE buffer per slice ([128, 9x1036] fp8) via 3 batched
   DMAs (tiles 1-7 in one strided descriptor set, edge tiles 0/8 separate),
   removing the per-tile dma_start issue serialization on the SP queue.
 - Every 128-row tile first runs the H (row) box pass as a banded matmul
   (lhsT[k,m]=1 iff m<=k<=m+8, zero-padded to 128 cols for FWL) over the
   fp8 input: PSUM f32, ACT drains to fp16 SBUF.
 - "Scan" tiles finish the W pass on the DVE with one tensor_tensor_scan
   (running 9-tap box, fp32 state); the batched output DMA reads the scan
   buffers directly.
 - "Direct" tiles compute the full 9x9 on the PE from raw fp8 input via
   shifted band matmuls (DoubleRow pair taps when USE_DR, else 9 plain
   taps), skipping the DVE entirely.
 - Output DMAs are batched per tile-kind per slice and issued from SP.
"""

import numpy as np

H = 1024
W = 1024
R = 4
D = 2 * R + 1
N_CORES = 8
SLICES_PER_CORE = 3
TILE = 120
N_TILES = 9
XW = 1036          # per-subtile pitch: 4 zeros | 1024 data | 8 pad
SXW = 9 * XW + 16  # slice input buffer width (extra room for last gap memset)
YW = 1040          # drained fp16 rows: 9 zeros | 1024 data | 4 zeros | 3 slack
SW = 1028          # scan free size

USE_DR = False     # DoubleRow rejects overlapping pair-taps (needs aligned
                   # pair stride); direct tiles use 9 plain fp8 taps

if USE_DR:
    DIRECT = (1, 3, 5, 7)
    SCAN_BATCH = (0, 2, 4, 6)
    SCAN_SINGLES = (8,)
else:
    DIRECT = (1, 3, 5)
    SCAN_BATCH = (0, 2, 4, 6)
    SCAN_SINGLES = (7, 8)

_COMPILED = {}


def _band_mid():
    """lhsT[k, m] = 1 iff m <= k <= m+8 (tile rows start at 120t-4);
    zero-padded to 128 output columns for FWL / DoubleRow stride rules."""
    k = np.arange(128)[:, None]
    m = np.arange(128)[None, :]
    return ((m <= k) & (k <= m + 2 * R) & (m < TILE)).astype(np.float32)


def _band_t0():
    """Tile-0 band for unshifted load (partition p = global row p, K=124):
    lhsT[k, m] = 1 iff m-4 <= k <= m+4, zero-padded to 128 columns."""
    k = np.arange(124)[:, None]
    m = np.arange(128)[None, :]
    return ((m - R <= k) & (k <= m + R) & (m < TILE)).astype(np.float32)


def _build():
    from concourse import bacc, mybir
    from concourse.tile import TileContext
    from concourse.ap import AP

    f8 = mybir.dt.float8e4
    f16 = mybir.dt.float16
    f32 = mybir.dt.float32
    nc = bacc.Bacc("TRN2", target_bir_lowering=False, debug=False,
                   num_devices=N_CORES)

    x = nc.dram_tensor("x", (SLICES_PER_CORE, H, W), f8,
                       kind="ExternalInput").ap()
    wp = nc.dram_tensor("wp", (128, 128), f8, kind="ExternalInput").ap()
    wp0 = nc.dram_tensor("wp0", (124, 128), f8, kind="ExternalInput").ap()
    wdr = nc.dram_tensor("wdr", (128, 2, 128), f8,
                         kind="ExternalInput").ap()
    wdr9 = nc.dram_tensor("wdr9", (128, 2, 128), f8,
                          kind="ExternalInput").ap()
    out = nc.dram_tensor("out", (SLICES_PER_CORE, H, W), f16,
                         kind="ExternalOutput").ap()

    add = mybir.AluOpType.add
    sub = mybir.AluOpType.subtract
    act_copy = mybir.ActivationFunctionType.Copy
    DRI = mybir.MatmulPerfMode.DoubleRowSwInterleave

    xh = x.tensor
    oh = out.tensor

    def kp_of(t):
        return 124 if t == 0 else (68 if t == 8 else 128)

    def m_of(t):
        return 64 if t == 8 else TILE

    with TileContext(nc) as tc:
        with tc.tile_pool(name="wts", bufs=1) as wpool, \
             tc.tile_pool(name="xp", bufs=1) as xpool, \
             tc.tile_pool(name="yb", bufs=1) as ypool, \
             tc.tile_pool(name="st", bufs=2) as spool, \
             tc.tile_pool(name="ob", bufs=2) as opool, \
             tc.tile_pool(name="ps", bufs=4, space="PSUM") as pspool:

            wp_t = wpool.tile([128, 128], f8)
            nc.sync.dma_start(wp_t[:], wp[:])
            wp0_t = wpool.tile([124, 128], f8)
            nc.sync.dma_start(wp0_t[:], wp0[:])
            wdr_t = wpool.tile([128, 2, 128], f8)
            nc.sync.dma_start(wdr_t[:], wdr[:])
            wdr9_t = wpool.tile([128, 2, 128], f8)
            nc.sync.dma_start(wdr9_t[:], wdr9[:])

            # one persistent input buffer per slice; zero the inter-tile
            # gap columns once (they serve as the W halo for edge taps)
            sxb = []
            for si in range(SLICES_PER_CORE):
                b = xpool.tile([128, SXW], f8, tag=f"sx{si}", name=f"sx{si}")
                sxb.append(b)
                nc.gpsimd.memset(b[:, 0:4], 0.0)
                gaps = AP(b[:, 0:1].tensor, b[:, 0:1].offset + 1028,
                          [[SXW, 128], [XW, 9], [1, 12]])
                nc.gpsimd.memset(gaps, 0.0)

            ybufs = [ypool.tile([TILE, YW], f16, tag=f"yb{i}",
                                name=f"yb{i}")
                     for i in range(5)]
            for yb in ybufs:
                nc.gpsimd.memset(yb[:, 0:D], 0.0)
                nc.gpsimd.memset(yb[:, D + W:YW], 0.0)
            yb_idx = 0

            for s in range(SLICES_PER_CORE):
                b = sxb[s]
                bh = b[:, 0:1].tensor
                boff = b[:, 0:1].offset

                # --- batched input DMAs: tiles 1..7, then edges 0 and 8 ---
                src_mid = AP(xh, s * H * W + (TILE - R) * W,
                             [[W, 128], [TILE * W, 7], [1, W]])
                dst_mid = AP(bh, boff + XW + 4, [[SXW, 128], [XW, 7], [1, W]])
                nc.sync.dma_start(dst_mid, src_mid)
                nc.sync.dma_start(b[0:124, 4:4 + W], x[s, 0:124, :])
                nc.sync.dma_start(b[0:68, 8 * XW + 4:8 * XW + 4 + W],
                                  x[s, 8 * TILE - R:H, :])

                def xv(t, a, bb, rows):
                    # 2D view of sub-tile t's columns [a, bb)
                    return b[0:rows, XW * t + a:XW * t + bb]

                st4 = spool.tile([TILE, len(SCAN_BATCH), SW], f16, tag="st4",
                                 name="st4")
                sts = {}
                for t in SCAN_SINGLES:
                    sts[t] = spool.tile([m_of(t), SW], f16, tag=f"sts{t}",
                                        name=f"sts{t}")
                ob = opool.tile([TILE, len(DIRECT), W], f16, tag="ob",
                                name="ob")

                order = [0, 2, 1, 4, 3, 6, 5, 7, 8] if USE_DR else \
                        [0, 2, 1, 4, 3, 6, 5, 7, 8]
                for t in order:
                    kp = kp_of(t)
                    m = m_of(t)
                    ps = pspool.tile([128, 1024], f32)
                    if t not in DIRECT:
                        # ---- H-pass band matmul, drain, DVE scan ----
                        lhs = wp0_t if t == 0 else wp_t
                        for hf in range(2):
                            w0 = 512 * hf
                            nc.tensor.matmul(
                                ps[:, w0:w0 + 512], lhs[0:kp, :],
                                xv(t, 4 + w0, 4 + w0 + 512, kp),
                                start=True, stop=True)
                        yb = ybufs[yb_idx % 5]
                        yb_idx += 1
                        nc.scalar.activation(yb[0:m, D:D + W], ps[0:m, :],
                                             act_copy)
                        if t in SCAN_SINGLES:
                            dst = sts[t][0:m, :]
                        else:
                            dst = st4[0:m, SCAN_BATCH.index(t), :]
                        nc.vector.tensor_tensor_scan(
                            dst, yb[0:m, D:D + SW], yb[0:m, 0:SW],
                            0.0, add, sub)
                    else:
                        # ---- direct 9x9 on the PE from raw fp8 ----
                        if USE_DR:
                            for hf in range(2):
                                w0 = 512 * hf
                                for p in range(5):
                                    lt = wdr_t if p < 4 else wdr9_t
                                    rhs = AP(bh,
                                             boff + XW * t + w0 + 2 * p,
                                             [[SXW, kp], [1, 512], [1, 2]])
                                    nc.tensor.matmul(
                                        ps[0:m, w0:w0 + 512],
                                        lt[0:kp, :, 0:m], rhs,
                                        start=(p == 0), stop=(p == 4),
                                        perf_mode=DRI)
                        else:
                            for hf in range(2):
                                w0 = 512 * hf
                                for j in range(D):
                                    nc.tensor.matmul(
                                        ps[:, w0:w0 + 512], wp_t[0:kp, :],
                                        xv(t, w0 + j, w0 + j + 512, kp),
                                        start=(j == 0), stop=(j == D - 1))
                        nc.scalar.activation(ob[0:m, DIRECT.index(t), :],
                                             ps[0:m, :], act_copy)

                # --- batched output DMAs (SP queue) ---
                base = s * H * W
                dst4 = AP(oh, base + TILE * SCAN_BATCH[0] * W,
                          [[W, TILE], [2 * TILE * W, len(SCAN_BATCH)],
                           [1, W]])
                nc.sync.dma_start(dst4, st4[0:TILE, 0:len(SCAN_BATCH),
                                            R:R + W])
                for t in SCAN_SINGLES:
                    m = m_of(t)
                    dst = AP(oh, base + TILE * t * W, [[W, m], [1, W]])
                    nc.sync.dma_start(dst, sts[t][0:m, R:R + W])
                dstd = AP(oh, base + TILE * DIRECT[0] * W,
                          [[W, TILE], [2 * TILE * W, len(DIRECT)], [1, W]])
                nc.sync.dma_start(dstd, ob[0:TILE, 0:len(DIRECT), :])

    nc.compile()
    return nc


def _get_nc():
    if "nc" not in _COMPILED:
        _COMPILED["nc"] = _build()
    return _COMPILED["nc"]


def _in_maps(x: np.ndarray):
    import ml_dtypes

    f8 = ml_dtypes.float8_e4m3fn
    xf = np.ascontiguousarray(np.asarray(x).astype(f8)).reshape(
        N_CORES * SLICES_PER_CORE, H, W)
    band = _band_mid()
    band0 = _band_t0()
    wp_np = band.astype(f8)
    wp0_np = band0.astype(f8)
    wdr_np = np.stack([band, band], axis=1).astype(f8)
    wdr9_np = np.stack([band, np.zeros_like(band)], axis=1).astype(f8)
    return [{
        "x": xf[c * SLICES_PER_CORE:(c + 1) * SLICES_PER_CORE],
        "wp": wp_np,
        "wp0": wp0_np,
        "wdr": wdr_np,
        "wdr9": wdr9_np,
    } for c in range(N_CORES)]


def kernel(x: np.ndarray) -> np.ndarray:
    from concourse.bass_utils import run_bass_kernel_spmd

    nc = _get_nc()
    res = run_bass_kernel_spmd(nc, _in_maps(x), core_ids=list(range(N_CORES)))
    outs = [res.results[c]["out"] for c in range(N_CORES)]
    return np.concatenate(outs, axis=0).reshape(8, 3, H, W).astype(np.float32)


# revision 24
# speedup vs baseline: 1.0215x; 1.0215x over previous
"""Trainium2 Bass kernel: separable box filter (radius 4) on (8,3,1024,1024) fp32.

v9: fp8 input, H-pass-first, paired PSUM drains, prefetch-everything.

 - Host casts x to fp8 e4m3 (halves input HBM traffic; L2 rel err 3.0e-3 vs
   the 2e-2 budget, measured on the true jax key-0 input).  Output fp16.
 - All 9 input DMAs (3 per slice: tiles 1-7 batched strided, edge tiles 0/8)
   are issued on SP before any compute-dependent DMA, so the input stream
   never blocks behind compute.
 - Per tile, the H (row) box pass is a banded matmul (lhsT[k,m]=1 iff
   m<=k<=m+8, zero-padded to 128 cols for FWL) over fp8: PSUM f32.
 - Tiles are processed in PAIRS sharing one [128,2048] PSUM tile (4 banks,
   ring 2 = 8 banks): one ACT activation drains both tiles (halves the ACT
   instruction count).
 - 18 "scan" tiles finish the W pass on the DVE (tensor_tensor_scan over
   the drained fp16 rows); 9 "direct" tiles ({1,3,5} per slice) compute the
   full 9x9 on the PE via 9 shifted band matmuls per half from raw fp8.
 - Scan groups are emitted before direct groups in each slice so the DVE
   is fed early while the PE grinds the direct taps.
 - Batched fp16 output DMAs (4-wide for scans, 3-wide for directs) on SP.
"""

import numpy as np

H = 1024
W = 1024
R = 4
D = 2 * R + 1
N_CORES = 8
SLICES_PER_CORE = 3
TILE = 120
N_TILES = 9
XW = 1036          # per-subtile pitch: 4 zeros | 1024 data | 8 pad
SXW = 9 * XW + 16  # slice input buffer width
YW = 1040          # drained fp16 rows: 9 zeros | 1024 data | 4 zeros | 3 slack
SW = 1028          # scan free size

DIRECT = (1, 3, 5)
# (kind, tiles) in per-slice emission order: scans first to feed the DVE
GROUPS = [("s", (0, 2)), ("s", (4, 6)), ("s", (7, 8)),
          ("d", (1, 3)), ("d", (5,))]

_COMPILED = {}


def _band_mid():
    """lhsT[k, m] = 1 iff m <= k <= m+8 (tile rows start at 120t-4);
    zero-padded to 128 output columns for FWL."""
    k = np.arange(128)[:, None]
    m = np.arange(128)[None, :]
    return ((m <= k) & (k <= m + 2 * R) & (m < TILE)).astype(np.float32)


def _band_t0():
    """Tile-0 band for unshifted load (partition p = global row p, K=124):
    lhsT[k, m] = 1 iff m-4 <= k <= m+4, zero-padded to 128 columns."""
    k = np.arange(124)[:, None]
    m = np.arange(128)[None, :]
    return ((m - R <= k) & (k <= m + R) & (m < TILE)).astype(np.float32)


def _build():
    from concourse import bacc, mybir
    from concourse.tile import TileContext
    from concourse.ap import AP

    f8 = mybir.dt.float8e4
    f16 = mybir.dt.float16
    f32 = mybir.dt.float32
    nc = bacc.Bacc("TRN2", target_bir_lowering=False, debug=False,
                   num_devices=N_CORES)

    x = nc.dram_tensor("x", (SLICES_PER_CORE, H, W), f8,
                       kind="ExternalInput").ap()
    wp = nc.dram_tensor("wp", (128, 128), f8, kind="ExternalInput").ap()
    wp0 = nc.dram_tensor("wp0", (124, 128), f8, kind="ExternalInput").ap()
    out = nc.dram_tensor("out", (SLICES_PER_CORE, H, W), f16,
                         kind="ExternalOutput").ap()

    add = mybir.AluOpType.add
    sub = mybir.AluOpType.subtract
    act_copy = mybir.ActivationFunctionType.Copy

    xh = x.tensor
    oh = out.tensor

    def kp_of(t):
        return 124 if t == 0 else (68 if t == 8 else 128)

    def m_of(t):
        return 64 if t == 8 else TILE

    with TileContext(nc) as tc:
        with tc.tile_pool(name="wts", bufs=1) as wpool, \
             tc.tile_pool(name="xp", bufs=1) as xpool, \
             tc.tile_pool(name="yb", bufs=1) as ypool, \
             tc.tile_pool(name="st", bufs=2) as spool, \
             tc.tile_pool(name="ob", bufs=2) as opool, \
             tc.tile_pool(name="ps", bufs=2, space="PSUM") as pspool:

            # --- input prefetch: slice buffers + all input DMAs first ---
            sxb = []
            for si in range(SLICES_PER_CORE):
                b = xpool.tile([128, SXW], f8, tag=f"sx{si}", name=f"sx{si}")
                sxb.append(b)
                nc.sync.dma_start(b[0:124, 4:4 + W], x[si, 0:124, :])
                if si == 0:
                    wp0_t = wpool.tile([124, 128], f8)
                    nc.sync.dma_start(wp0_t[:], wp0[:])
                    wp_t = wpool.tile([128, 128], f8)
                    nc.sync.dma_start(wp_t[:], wp[:])
                src_mid = AP(xh, si * H * W + (TILE - R) * W,
                             [[W, 128], [TILE * W, 7], [1, W]])
                dst_mid = AP(b[:, 0:1].tensor, b[:, 0:1].offset + XW + 4,
                             [[SXW, 128], [XW, 7], [1, W]])
                nc.sync.dma_start(dst_mid, src_mid)
                nc.sync.dma_start(b[0:68, 8 * XW + 4:8 * XW + 4 + W],
                                  x[si, 8 * TILE - R:H, :])
                nc.gpsimd.memset(b[:, 0:4], 0.0)
                gaps = AP(b[:, 0:1].tensor, b[:, 0:1].offset + 1028,
                          [[SXW, 128], [XW, 9], [1, 12]])
                nc.gpsimd.memset(gaps, 0.0)

            # persistent paired drain buffers (zeroed scan pads)
            yb2s = []
            for i in range(3):
                yb2 = ypool.tile([TILE, 2, YW], f16, tag=f"yb{i}",
                                 name=f"yb{i}")
                yb2s.append(yb2)
                padl = AP(yb2[:, 0:1, 0:1].tensor, yb2[:, 0:1, 0:1].offset,
                          [[2 * YW, TILE], [YW, 2], [1, D]])
                nc.gpsimd.memset(padl, 0.0)
                padr = AP(yb2[:, 0:1, 0:1].tensor,
                          yb2[:, 0:1, 0:1].offset + D + W,
                          [[2 * YW, TILE], [YW, 2], [1, YW - D - W]])
                nc.gpsimd.memset(padr, 0.0)
            ygi = 0

            for s in range(SLICES_PER_CORE):
                b = sxb[s]
                bh = b[:, 0:1].tensor
                boff = b[:, 0:1].offset

                def xv(t, a, bb, rows):
                    return b[0:rows, XW * t + a:XW * t + bb]

                st4 = spool.tile([TILE, 4, SW], f16, tag="st4", name="st4")
                st7 = spool.tile([TILE, SW], f16, tag="st7", name="st7")
                st8 = spool.tile([64, SW], f16, tag="st8", name="st8")
                ob = opool.tile([TILE, 3, W], f16, tag="ob", name="ob")

                for kind, tiles in GROUPS:
                    ps = pspool.tile([128, 2048], f32)
                    for gi, t in enumerate(tiles):
                        kp = kp_of(t)
                        base = gi * 1024
                        lhs = wp0_t if t == 0 else wp_t
                        if kind == "s":
                            for hf in range(2):
                                w0 = 512 * hf
                                nc.tensor.matmul(
                                    ps[:, base + w0:base + w0 + 512],
                                    lhs[0:kp, :],
                                    xv(t, 4 + w0, 4 + w0 + 512, kp),
                                    start=True, stop=True)
                        else:
                            for hf in range(2):
                                w0 = 512 * hf
                                for j in range(D):
                                    nc.tensor.matmul(
                                        ps[:, base + w0:base + w0 + 512],
                                        wp_t[0:kp, :],
                                        xv(t, w0 + j, w0 + j + 512, kp),
                                        start=(j == 0), stop=(j == D - 1))
                    # one drain for the whole group
                    if kind == "s":
                        yb2 = yb2s[ygi % 3]
                        ygi += 1
                        dst = AP(yb2[:, 0:1, 0:1].tensor,
                                 yb2[:, 0:1, 0:1].offset + D,
                                 [[2 * YW, TILE], [YW, len(tiles)], [1, W]])
                        nc.scalar.activation(dst, ps[0:TILE, 0:1024 *
                                                     len(tiles)], act_copy)
                        for gi, t in enumerate(tiles):
                            m = m_of(t)
                            if t in (0, 2, 4, 6):
                                dst_s = st4[0:m, (0, 2, 4, 6).index(t), :]
                            elif t == 7:
                                dst_s = st7[0:m, :]
                            else:
                                dst_s = st8[0:m, :]
                            nc.vector.tensor_tensor_scan(
                                dst_s, yb2[0:m, gi, D:D + SW],
                                yb2[0:m, gi, 0:SW], 0.0, add, sub)
                    else:
                        oslots = [DIRECT.index(t) for t in tiles]
                        dst = AP(ob[:, 0:1, 0:1].tensor,
                                 ob[:, 0:1, 0:1].offset + oslots[0] * W,
                                 [[3 * W, TILE], [W, len(tiles)], [1, W]])
                        nc.scalar.activation(dst, ps[0:TILE, 0:1024 *
                                                     len(tiles)], act_copy)

                # --- batched output DMAs (SP) ---
                base = s * H * W
                dst4 = AP(oh, base, [[W, TILE], [2 * TILE * W, 4], [1, W]])
                nc.sync.dma_start(dst4, st4[0:TILE, 0:4, R:R + W])
                dst7 = AP(oh, base + 7 * TILE * W, [[W, TILE], [1, W]])
                nc.sync.dma_start(dst7, st7[0:TILE, R:R + W])
                dst8 = AP(oh, base + 8 * TILE * W, [[W, 64], [1, W]])
                nc.sync.dma_start(dst8, st8[0:64, R:R + W])
                dstd = AP(oh, base + TILE * W,
                          [[W, TILE], [2 * TILE * W, 3], [1, W]])
                nc.sync.dma_start(dstd, ob[0:TILE, 0:3, :])

    nc.compile()
    return nc


def _get_nc():
    if "nc" not in _COMPILED:
        _COMPILED["nc"] = _build()
    return _COMPILED["nc"]


def _in_maps(x: np.ndarray):
    import ml_dtypes

    f8 = ml_dtypes.float8_e4m3fn
    xf = np.ascontiguousarray(np.asarray(x).astype(f8)).reshape(
        N_CORES * SLICES_PER_CORE, H, W)
    return [{
        "x": xf[c * SLICES_PER_CORE:(c + 1) * SLICES_PER_CORE],
        "wp": _band_mid().astype(f8),
        "wp0": _band_t0().astype(f8),
    } for c in range(N_CORES)]


def kernel(x: np.ndarray) -> np.ndarray:
    from concourse.bass_utils import run_bass_kernel_spmd

    nc = _get_nc()
    res = run_bass_kernel_spmd(nc, _in_maps(x), core_ids=list(range(N_CORES)))
    outs = [res.results[c]["out"] for c in range(N_CORES)]
    return np.concatenate(outs, axis=0).reshape(8, 3, H, W).astype(np.float32)
